# revision 7
# baseline (speedup 1.0000x reference)
"""Trainium2 Bass kernel for nn_BinaryXnorExceptOutliersLinear.

Reference computation (fp32):
    mean = mean(W); std = std(W, ddof=1)
    outliers = (W < mean - 1.96 std) | (W > mean + 1.96 std)
    scale = sum(|W| * ~outliers) / sum(~outliers)            # scalar
    W_bin = where(outliers, W, W * scale)
    out   = x @ W_bin.T + bias                               # x: [4, 2048, 4096]

Distribution strategy (8 NeuronCores, full inputs in / full output out):
  * Weight-stat passes are sharded over W rows (512 rows/core). The
    mean/std partial sums are all-reduced on the host between launches
    (a few dozen floats); the scale partial sums are all-reduced on
    device (gpsimd collective AllReduce) inside the fused apply kernel.
  * The GEMM is data-parallel over x rows (1024 rows/core); every core
    reads the full binarized weight (bf16) and computes its slice of the
    output.

Three SPMD launches (identical NEFF on cores 0-7, different per-core data):
  L1 stats1: per-core partial [sum W, sum W^2, sum |W|] over its W shard.
  L2 apply:  given (lower, upper, sum|W|): builds the outlier mask for its
             shard, partial [sum |W*outlier|, count] -> partition_all_reduce
             -> cross-core AllReduce -> scale on device -> W_bin shard in
             bf16. Also casts its x row-shard to bf16 via SWDGE cast-DMA.
  L3 gemm:   out_shard = x_shard @ W_bin^T + bias, bf16 matmuls with fp32
             PSUM accumulation; operand tiles are produced with DMA
             transpose (contraction dim on partitions).

All NEFFs are input-independent (runtime scalars enter as small tensors),
so the neuron compile cache stays warm across runs.
"""

import numpy as np
import ml_dtypes

import concourse.bass as bass
import concourse.mybir as mybir
import concourse.tile as tile
from concourse import bacc
from concourse import bass_isa
from concourse.bass_utils import run_bass_kernel_spmd
from concourse.kernels.tile_matmul import (
    composable_matmul_tile_kernel,
    dma_from_dram_kxm,
    dma_from_dram_kxn,
    dma_to_dram_mxn,
)

F32 = mybir.dt.float32
BF16 = mybir.dt.bfloat16
ALU = mybir.AluOpType

N_CORES = 8
B, S, D = 4, 2048, 4096  # x: [B, S, D]
O = 4096                 # weight: [O, D]
R_TOT = B * S            # 8192 rows total
R = R_TOT // N_CORES     # 1024 x-rows per core
OSH = O // N_CORES       # 512 weight rows per core (stat shards)
PO_W = OSH // 128        # 4 partition-chunks per W shard
PO_X = R // 128          # 8 partition-chunks per x shard
N_ELEM = float(O * D)
NSIG = 1.96

_CACHE = {}
LAST_TIMINGS = []  # (name, exec_time_ns) per launch when BASS_TRACE=1


def _wview(ap):
    return ap.rearrange("(po pi) f -> pi po f", pi=128)


# ---------------------------------------------------------------- L1: stats1
def _build_stats1():
    nc = bacc.Bacc(None, target_bir_lowering=False)
    w_sh = nc.declare_dram_parameter("w_sh", [OSH, D], F32, isOutput=False)
    st_o = nc.declare_dram_parameter("stats1", [128, 3 * PO_W], F32, isOutput=True)

    with tile.TileContext(nc) as tc:
        with (
            tc.tile_pool(name="wp", bufs=1) as wp,
            tc.tile_pool(name="scr", bufs=2) as scr,
        ):
            wv = _wview(w_sh)
            st = wp.tile([128, 3 * PO_W], F32)
            for c in range(PO_W):
                wt = scr.tile([128, D], F32, tag="wt")
                nc.sync.dma_start(wt[:], wv[:, c])
                # sum W (exact fp32 tree on DVE; feeds the outlier threshold)
                nc.vector.tensor_reduce(
                    st[:, c : c + 1], wt[:], axis=mybir.AxisListType.X, op=ALU.add
                )
                # sum W^2: ACT squares, DVE reduces (threshold-sensitive too)
                sq = scr.tile([128, D], F32, tag="sq")
                nc.scalar.activation(sq[:], wt[:], mybir.ActivationFunctionType.Square)
                nc.vector.tensor_reduce(
                    st[:, PO_W + c : PO_W + c + 1],
                    sq[:],
                    axis=mybir.AxisListType.X,
                    op=ALU.add,
                )
                # sum |W| (only feeds `scale`; ACT accumulate is plenty accurate)
                dm = scr.tile([128, D], BF16, tag="dm")
                nc.scalar.activation(
                    dm[:],
                    wt[:],
                    mybir.ActivationFunctionType.Abs,
                    accum_out=st[:, 2 * PO_W + c : 2 * PO_W + c + 1],
                )
            nc.sync.dma_start(st_o[:], st[:])
    nc.compile()
    return nc


# ------------------------------------------------------- L2: fused apply
def _build_apply():
    nc = bacc.Bacc(None, target_bir_lowering=False)
    w_sh = nc.declare_dram_parameter("w_sh", [OSH, D], F32, isOutput=False)
    x_sh = nc.declare_dram_parameter("x_sh", [R, D], F32, isOutput=False)
    bnd = nc.declare_dram_parameter("bnd", [128, 4], F32, isOutput=False)
    wb_o = nc.declare_dram_parameter("wbin", [OSH, D], BF16, isOutput=True)
    xb_o = nc.declare_dram_parameter("xb", [R, D], BF16, isOutput=True)

    cc_in = nc.dram_tensor("cc_in", [128, 2], F32)
    cc_out = nc.dram_tensor("cc_out", [128, 2], F32, addr_space="Shared")

    with tile.TileContext(nc) as tc:
        with (
            tc.tile_pool(name="cn", bufs=1) as cn,
            tc.tile_pool(name="wp", bufs=1) as wp,
            tc.tile_pool(name="ch", bufs=2) as ch,
            tc.tile_pool(name="xp", bufs=2) as xp,
        ):
            bt = cn.tile([128, 4], F32)
            nc.sync.dma_start(bt[:], bnd[:])
            lo_ap = bt[:, 0:1]
            hi_ap = bt[:, 1:2]
            sabs_ap = bt[:, 2:3]

            wt = wp.tile([128, PO_W, D], F32)
            wv = _wview(w_sh)
            # outlier mask; 0/1 is exact in bf16 and halves SBUF footprint
            mko = wp.tile([128, PO_W, D], BF16)
            acc = cn.tile([128, 2 * PO_W], F32)  # [sum|w*out| chunks, cnt chunks]

            for c in range(PO_W):
                nc.sync.dma_start(wt[:, c], wv[:, c])
                mhi = ch.tile([128, D], F32, tag="mhi")
                nc.vector.tensor_scalar(mhi[:], wt[:, c], hi_ap, None, ALU.is_gt)
                # outlier = (w < lo) + (w > hi), count fused into accum
                nc.vector.scalar_tensor_tensor(
                    mko[:, c],
                    wt[:, c],
                    lo_ap,
                    mhi[:],
                    ALU.is_lt,
                    ALU.add,
                    accum_out=acc[:, PO_W + c : PO_W + c + 1],
                )
                # |w| on ACT, then sum |w|*outlier fused on DVE
                aw = ch.tile([128, D], BF16, tag="aw")
                nc.scalar.activation(aw[:], wt[:, c], mybir.ActivationFunctionType.Abs)
                dm = ch.tile([128, D], BF16, tag="wbdm")
                nc.vector.scalar_tensor_tensor(
                    dm[:],
                    aw[:],
                    1.0,
                    mko[:, c],
                    ALU.mult,
                    ALU.mult,
                    accum_out=acc[:, c : c + 1],
                )

            # combine chunks -> [128, 2] (sum|w*out|, cnt)
            part = cn.tile([128, 2], F32)
            nc.vector.tensor_reduce(
                part[:, 0:1], acc[:, 0:PO_W], axis=mybir.AxisListType.X, op=ALU.add
            )
            nc.vector.tensor_reduce(
                part[:, 1:2],
                acc[:, PO_W : 2 * PO_W],
                axis=mybir.AxisListType.X,
                op=ALU.add,
            )
            # total over partitions, then over cores
            red = cn.tile([128, 2], F32)
            nc.gpsimd.partition_all_reduce(
                red[:], part[:], channels=128, reduce_op=bass_isa.ReduceOp.add
            )
            nc.gpsimd.dma_start(cc_in[:], red[:])
            nc.gpsimd.collective_compute(
                "AllReduce",
                ALU.add,
                replica_groups=[list(range(N_CORES))],
                ins=[cc_in[:]],
                outs=[cc_out[:]],
            )
            tot = cn.tile([128, 2], F32)
            nc.sync.dma_start(tot[:], cc_out[:])

            # scale = (sum|W| - sum|W*out|) / (N - cnt); also keep 1 - scale
            sc = cn.tile([128, 4], F32)
            nc.vector.scalar_tensor_tensor(
                sc[:, 0:1], tot[:, 0:1], -1.0, sabs_ap, ALU.mult, ALU.add
            )  # num = sabs - absout
            nc.vector.tensor_scalar(
                sc[:, 1:2], tot[:, 1:2], -1.0, N_ELEM, ALU.mult, ALU.add
            )  # den = N - cnt
            nc.vector.reciprocal(sc[:, 2:3], sc[:, 1:2])
            nc.vector.tensor_tensor(sc[:, 0:1], sc[:, 0:1], sc[:, 2:3], ALU.mult)
            nc.vector.tensor_scalar(
                sc[:, 1:2], sc[:, 0:1], -1.0, 1.0, ALU.mult, ALU.add
            )  # 1 - scale
            sc0 = sc[:, 0:1]
            sc1 = sc[:, 1:2]

            # W_bin = W * (outlier*(1-scale) + scale), cast bf16
            bv = _wview(wb_o)
            for c in range(PO_W):
                f = ch.tile([128, D], F32, tag="mhi")
                nc.vector.tensor_scalar(f[:], mko[:, c], sc1, sc0, ALU.mult, ALU.add)
                wb = ch.tile([128, D], BF16, tag="wbdm")
                nc.vector.tensor_tensor(wb[:], wt[:, c], f[:], ALU.mult)
                nc.sync.dma_start(bv[:, c], wb[:])

            # x -> bf16 purely in DMA (SWDGE cast on load, HWDGE store)
            xv = _wview(x_sh)
            xbv = _wview(xb_o)
            for c in range(PO_X):
                xt = xp.tile([128, D], BF16, tag="xt")
                nc.gpsimd.dma_start(xt[:], xv[:, c])
                nc.sync.dma_start(xbv[:, c], xt[:])
    nc.compile()
    return nc


# ---------------------------------------------------------------- L3: gemm
def _build_gemm():
    nc = bacc.Bacc(None, target_bir_lowering=False)
    xb = nc.declare_dram_parameter("xb", [R, D], BF16, isOutput=False)
    wb = nc.declare_dram_parameter("wbin", [O, D], BF16, isOutput=False)
    br = nc.declare_dram_parameter("bias_rep", [128, O], F32, isOutput=False)
    out = nc.declare_dram_parameter("out", [R, O], F32, isOutput=True)

    with tile.TileContext(nc) as tc:
        import contextlib

        with contextlib.ExitStack() as ctx:
            cn = ctx.enter_context(tc.tile_pool(name="const", bufs=1))
            kxm_pool = ctx.enter_context(tc.tile_pool(name="kxm_pool", bufs=9))
            kxn_pool = ctx.enter_context(tc.tile_pool(name="kxn_pool", bufs=9))

            bias_sb = cn.tile([128, O], F32)
            nc.sync.dma_start(bias_sb[:], br[:])

            # kxm = x^T tiles [k, m] from x [R, D] (transpose_ap);
            # kxn = W_bin^T tiles [k, n] from W_bin [O, D] (transpose_ap).
            kxm_producer, kxm_shape = dma_from_dram_kxm(
                kxm_pool, xb[:], transpose_ap=True
            )
            kxn_producer, kxn_shape = dma_from_dram_kxn(
                kxn_pool, wb[:], transpose_ap=True
            )
            mxn_consumer = dma_to_dram_mxn(out[:])

            def reducer(nc, psum, sbuf, md):
                n0 = md.n_tile_idx * md.n_tile + md.n_subtile_idx * md.n_subtile
                nss = md.n_slice_size
                nc.vector.tensor_tensor(
                    sbuf[:, 0, :nss],
                    psum[:, :nss],
                    bias_sb[:, n0 : n0 + nss],
                    ALU.add,
                )

            composable_matmul_tile_kernel(
                tc,
                kxm_shape=kxm_shape,
                kxn_shape=kxn_shape,
                output_type=F32,
                kxm_producer=kxm_producer,
                kxn_producer=kxn_producer,
                mxn_consumer=mxn_consumer,
                mxn_subtile_reducer=reducer,
                psum_n_bufs=2,
                cache_tiles=True,
            )
    nc.compile()
    return nc


def _get(name, builder):
    if name not in _CACHE:
        _CACHE[name] = builder()
    return _CACHE[name]


def _run(name, nc, in_maps, cores):
    r = run_bass_kernel_spmd(nc, in_maps, core_ids=cores)
    if r.exec_time_ns is not None:
        LAST_TIMINGS.append((name, r.exec_time_ns))
    return r.results


# ---------------------------------------------------------------- driver
def kernel(x: np.ndarray, weight: np.ndarray, bias: np.ndarray) -> np.ndarray:
    LAST_TIMINGS.clear()
    x2 = np.ascontiguousarray(x.reshape(R_TOT, D))
    w = np.ascontiguousarray(weight)
    cores = list(range(N_CORES))
    w_shards = [np.ascontiguousarray(w[c * OSH : (c + 1) * OSH]) for c in cores]
    x_shards = [np.ascontiguousarray(x2[c * R : (c + 1) * R]) for c in cores]

    # L1: partial sums
    nc1 = _get("stats1", _build_stats1)
    r1 = _run("stats1", nc1, [{"w_sh": w_shards[c]} for c in cores], cores)
    st1 = np.stack([r1[c]["stats1"] for c in cores]).astype(np.float64)

    s_w = st1[:, :, 0:PO_W].sum()
    s_w2 = st1[:, :, PO_W : 2 * PO_W].sum()
    s_abs = st1[:, :, 2 * PO_W : 3 * PO_W].sum()
    mean = s_w / N_ELEM
    var = (s_w2 - N_ELEM * mean * mean) / (N_ELEM - 1.0)
    std = np.sqrt(var)
    lower = mean - NSIG * std
    upper = mean + NSIG * std

    # L2: masks + device-side scale + W_bin + x cast
    nc2 = _get("apply", _build_apply)
    bnd = np.zeros((128, 4), np.float32)
    bnd[:, 0] = lower
    bnd[:, 1] = upper
    # partition_all_reduce + AllReduce sum 128 partitions x 8 cores of the
    # per-core [128,2] partials; sabs enters once per lane, so no scaling
    bnd[:, 2] = s_abs
    r2 = _run(
        "apply",
        nc2,
        [{"w_sh": w_shards[c], "x_sh": x_shards[c], "bnd": bnd} for c in cores],
        cores,
    )
    wbin = np.ascontiguousarray(
        np.concatenate([r2[c]["wbin"] for c in cores], axis=0)
    )
    xb_shards = [r2[c]["xb"] for c in cores]

    # L3: data-parallel GEMM
    nc3 = _get("gemm", _build_gemm)
    bias_rep = np.ascontiguousarray(
        np.broadcast_to(bias.astype(np.float32), (128, O))
    )
    r3 = _run(
        "gemm",
        nc3,
        [{"xb": xb_shards[c], "wbin": wbin, "bias_rep": bias_rep} for c in cores],
        cores,
    )
    out = np.concatenate([r3[c]["out"] for c in cores], axis=0)
    return out.reshape(B, S, O)


# revision 12
# speedup vs baseline: 1.0898x; 1.0898x over previous
"""Trainium2 Bass kernel for nn_BinaryXnorExceptOutliersLinear.

Reference computation (fp32):
    mean = mean(W); std = std(W, ddof=1)
    outliers = (W < mean - 1.96 std) | (W > mean + 1.96 std)
    scale = sum(|W| * ~outliers) / sum(~outliers)            # scalar
    W_bin = where(outliers, W, W * scale)
    out   = x @ W_bin.T + bias                               # x: [4, 2048, 4096]

Distribution strategy (8 NeuronCores, full inputs in / full output out):
  * Weight-stat passes are sharded over W rows (512 rows/core). The global
    scalars (mean/std bounds, then scale) are reduced on the host between
    launches -- a few dozen floats each time. Host-mediated beats an
    on-device collective here: PJRT dispatch skews core start times by
    tens of us, and a mid-kernel AllReduce pulls that skew into every
    core's measured span.
  * The GEMM is data-parallel over x rows (1024 rows/core); every core
    reads the full binarized weight (bf16) and computes its slice of the
    output.

Four SPMD launches (identical NEFF on cores 0-7, different per-core data):
  L1 stats1:    per-core partial [sum W, sum W^2, sum |W|] over its W shard.
  L2 maskstats: given (lower, upper): outlier mask shard (bf16 0/1) and
                partial [sum |W*outlier|, count], with the count and masked
                sums fused into the mask-building DVE ops.
  L3 apply:     given scale: W_bin = W * (outlier*(1-scale) + scale) in
                bf16; also casts its x row-shard to bf16.
  L4 gemm:      out_shard = x_shard @ W_bin^T + bias, bf16 matmuls with
                fp32 PSUM accumulation; operand tiles are produced with
                DMA transpose (contraction dim on partitions).

All NEFFs are input-independent (runtime scalars enter as small tensors),
so the neuron compile cache stays warm across runs.
"""

import numpy as np
import ml_dtypes

import concourse.bass as bass
import concourse.mybir as mybir
import concourse.tile as tile
from concourse import bacc
from concourse.bass_utils import run_bass_kernel_spmd
from concourse.kernels.tile_matmul import (
    composable_matmul_tile_kernel,
    dma_from_dram_kxm,
    dma_from_dram_kxn,
    dma_to_dram_mxn,
)

F32 = mybir.dt.float32
BF16 = mybir.dt.bfloat16
ALU = mybir.AluOpType

N_CORES = 8
B, S, D = 4, 2048, 4096  # x: [B, S, D]
O = 4096                 # weight: [O, D]
R_TOT = B * S            # 8192 rows total
R = R_TOT // N_CORES     # 1024 x-rows per core
OSH = O // N_CORES       # 512 weight rows per core (stat shards)
PO_W = OSH // 128        # 4 partition-chunks per W shard
PO_X = R // 128          # 8 partition-chunks per x shard
N_ELEM = float(O * D)
NSIG = 1.96

_CACHE = {}
LAST_TIMINGS = []  # (name, exec_time_ns) per launch when BASS_TRACE=1


def _wview(ap):
    return ap.rearrange("(po pi) f -> pi po f", pi=128)


# ---------------------------------------------------------------- L1: stats1
def _build_stats1():
    nc = bacc.Bacc(None, target_bir_lowering=False)
    w_sh = nc.declare_dram_parameter("w_sh", [OSH, D], F32, isOutput=False)
    st_o = nc.declare_dram_parameter("stats1", [128, 3 * PO_W], F32, isOutput=True)

    with tile.TileContext(nc) as tc:
        with (
            tc.tile_pool(name="wp", bufs=1) as wp,
            tc.tile_pool(name="scr", bufs=2) as scr,
        ):
            wv = _wview(w_sh)
            st = wp.tile([128, 3 * PO_W], F32)
            for c in range(PO_W):
                wt = scr.tile([128, D], F32, tag="wt")
                nc.sync.dma_start(wt[:], wv[:, c])
                # sum W (exact fp32 tree on DVE; feeds the outlier threshold)
                nc.vector.tensor_reduce(
                    st[:, c : c + 1], wt[:], axis=mybir.AxisListType.X, op=ALU.add
                )
                # sum W^2: ACT squares, DVE reduces (threshold-sensitive too)
                sq = scr.tile([128, D], F32, tag="sq")
                nc.scalar.activation(sq[:], wt[:], mybir.ActivationFunctionType.Square)
                nc.vector.tensor_reduce(
                    st[:, PO_W + c : PO_W + c + 1],
                    sq[:],
                    axis=mybir.AxisListType.X,
                    op=ALU.add,
                )
                # sum |W| (only feeds `scale`; ACT accumulate is plenty accurate)
                dm = scr.tile([128, D], BF16, tag="dm")
                nc.scalar.activation(
                    dm[:],
                    wt[:],
                    mybir.ActivationFunctionType.Abs,
                    accum_out=st[:, 2 * PO_W + c : 2 * PO_W + c + 1],
                )
            nc.sync.dma_start(st_o[:], st[:])
    nc.compile()
    return nc


# ------------------------------------------------------- L2: mask + stats2
def _build_maskstats():
    nc = bacc.Bacc(None, target_bir_lowering=False)
    w_sh = nc.declare_dram_parameter("w_sh", [OSH, D], F32, isOutput=False)
    bnd = nc.declare_dram_parameter("bnd", [128, 2], F32, isOutput=False)
    st_o = nc.declare_dram_parameter("stats2", [128, 2 * PO_W], F32, isOutput=True)
    mk_o = nc.declare_dram_parameter("mask", [OSH, D], BF16, isOutput=True)

    with tile.TileContext(nc) as tc:
        with (
            tc.tile_pool(name="cn", bufs=1) as cn,
            tc.tile_pool(name="ch", bufs=2) as ch,
        ):
            bt = cn.tile([128, 2], F32)
            nc.sync.dma_start(bt[:], bnd[:])
            lo_ap = bt[:, 0:1]
            hi_ap = bt[:, 1:2]
            st = cn.tile([128, 2 * PO_W], F32)
            wv = _wview(w_sh)
            mv = _wview(mk_o)
            for c in range(PO_W):
                wt = ch.tile([128, D], F32, tag="wt")
                nc.sync.dma_start(wt[:], wv[:, c])
                mhi = ch.tile([128, D], F32, tag="mhi")
                nc.vector.tensor_scalar(mhi[:], wt[:], hi_ap, None, ALU.is_gt)
                # outlier = (w < lo) + (w > hi); count fused into accum
                mko = ch.tile([128, D], BF16, tag="mko")
                nc.vector.scalar_tensor_tensor(
                    mko[:],
                    wt[:],
                    lo_ap,
                    mhi[:],
                    ALU.is_lt,
                    ALU.add,
                    accum_out=st[:, PO_W + c : PO_W + c + 1],
                )
                # |w| on ACT, then sum |w|*outlier fused on DVE
                aw = ch.tile([128, D], BF16, tag="aw")
                nc.scalar.activation(aw[:], wt[:], mybir.ActivationFunctionType.Abs)
                dm = ch.tile([128, D], BF16, tag="dm")
                nc.vector.scalar_tensor_tensor(
                    dm[:],
                    aw[:],
                    1.0,
                    mko[:],
                    ALU.mult,
                    ALU.mult,
                    accum_out=st[:, c : c + 1],
                )
                nc.sync.dma_start(mv[:, c], mko[:])
            nc.sync.dma_start(st_o[:], st[:])
    nc.compile()
    return nc


# ---------------------------------------------------------------- L3: apply
def _build_apply():
    nc = bacc.Bacc(None, target_bir_lowering=False)
    w_sh = nc.declare_dram_parameter("w_sh", [OSH, D], F32, isOutput=False)
    mk_i = nc.declare_dram_parameter("mask", [OSH, D], BF16, isOutput=False)
    sc_i = nc.declare_dram_parameter("sc", [128, 2], F32, isOutput=False)
    x_sh = nc.declare_dram_parameter("x_sh", [R, D], F32, isOutput=False)
    wb_o = nc.declare_dram_parameter("wbin", [OSH, D], BF16, isOutput=True)
    xb_o = nc.declare_dram_parameter("xb", [R, D], BF16, isOutput=True)

    with tile.TileContext(nc) as tc:
        with (
            tc.tile_pool(name="cn", bufs=1) as cn,
            tc.tile_pool(name="ch", bufs=2) as ch,
            tc.tile_pool(name="xp", bufs=3) as xp,
        ):
            sct = cn.tile([128, 2], F32)
            nc.sync.dma_start(sct[:], sc_i[:])
            sc0 = sct[:, 0:1]  # scale
            sc1 = sct[:, 1:2]  # 1 - scale
            wv = _wview(w_sh)
            mv = _wview(mk_i)
            bv = _wview(wb_o)
            for c in range(PO_W):
                wt = ch.tile([128, D], F32, tag="wt")
                mk = ch.tile([128, D], BF16, tag="mk")
                nc.sync.dma_start(wt[:], wv[:, c])
                nc.sync.dma_start(mk[:], mv[:, c])
                # f = outlier*(1-scale) + scale -> 1 for outliers, else scale
                f = ch.tile([128, D], F32, tag="f")
                nc.vector.tensor_scalar(f[:], mk[:], sc1, sc0, ALU.mult, ALU.add)
                wb = ch.tile([128, D], BF16, tag="wb")
                nc.vector.tensor_tensor(wb[:], wt[:], f[:], ALU.mult)
                nc.sync.dma_start(bv[:, c], wb[:])

            # x -> bf16 (DVE cast copies; DMA on the scalar HWDGE queue)
            HD = D // 2
            xv = x_sh.rearrange("(po pi) (fo fi) -> pi po fo fi", pi=128, fi=HD)
            xbv = xb_o.rearrange("(po pi) (fo fi) -> pi po fo fi", pi=128, fi=HD)
            for c in range(2 * PO_X):
                xt = xp.tile([128, HD], F32, tag="xt")
                nc.scalar.dma_start(xt[:], xv[:, c // 2, c % 2])
                xbt = xp.tile([128, HD], BF16, tag="xbt")
                nc.vector.tensor_copy(xbt[:], xt[:])
                nc.scalar.dma_start(xbv[:, c // 2, c % 2], xbt[:])
    nc.compile()
    return nc


# ---------------------------------------------------------------- L4: gemm
def _build_gemm():
    nc = bacc.Bacc(None, target_bir_lowering=False)
    xb = nc.declare_dram_parameter("xb", [R, D], BF16, isOutput=False)
    wb = nc.declare_dram_parameter("wbin", [O, D], BF16, isOutput=False)
    br = nc.declare_dram_parameter("bias_rep", [128, O], F32, isOutput=False)
    out = nc.declare_dram_parameter("out", [R, O], F32, isOutput=True)

    with tile.TileContext(nc) as tc:
        import contextlib

        with contextlib.ExitStack() as ctx:
            cn = ctx.enter_context(tc.tile_pool(name="const", bufs=1))
            kxm_pool = ctx.enter_context(tc.tile_pool(name="kxm_pool", bufs=9))
            kxn_pool = ctx.enter_context(tc.tile_pool(name="kxn_pool", bufs=9))

            bias_sb = cn.tile([128, O], F32)
            nc.sync.dma_start(bias_sb[:], br[:])

            # kxm = x^T tiles [k, m] from x [R, D] (transpose_ap);
            # kxn = W_bin^T tiles [k, n] from W_bin [O, D] (transpose_ap).
            kxm_producer, kxm_shape = dma_from_dram_kxm(
                kxm_pool, xb[:], transpose_ap=True
            )
            kxn_producer, kxn_shape = dma_from_dram_kxn(
                kxn_pool, wb[:], transpose_ap=True
            )
            mxn_consumer = dma_to_dram_mxn(out[:])

            def reducer(nc, psum, sbuf, md):
                n0 = md.n_tile_idx * md.n_tile + md.n_subtile_idx * md.n_subtile
                nss = md.n_slice_size
                nc.vector.tensor_tensor(
                    sbuf[:, 0, :nss],
                    psum[:, :nss],
                    bias_sb[:, n0 : n0 + nss],
                    ALU.add,
                )

            composable_matmul_tile_kernel(
                tc,
                kxm_shape=kxm_shape,
                kxn_shape=kxn_shape,
                output_type=F32,
                kxm_producer=kxm_producer,
                kxn_producer=kxn_producer,
                mxn_consumer=mxn_consumer,
                mxn_subtile_reducer=reducer,
                psum_n_bufs=2,
                cache_tiles=True,
            )
    nc.compile()
    return nc


def _get(name, builder):
    if name not in _CACHE:
        _CACHE[name] = builder()
    return _CACHE[name]


def _run(name, nc, in_maps, cores):
    r = run_bass_kernel_spmd(nc, in_maps, core_ids=cores)
    if r.exec_time_ns is not None:
        LAST_TIMINGS.append((name, r.exec_time_ns))
    return r.results


# ---------------------------------------------------------------- driver
def kernel(x: np.ndarray, weight: np.ndarray, bias: np.ndarray) -> np.ndarray:
    LAST_TIMINGS.clear()
    x2 = np.ascontiguousarray(x.reshape(R_TOT, D))
    w = np.ascontiguousarray(weight)
    cores = list(range(N_CORES))
    w_shards = [np.ascontiguousarray(w[c * OSH : (c + 1) * OSH]) for c in cores]
    x_shards = [np.ascontiguousarray(x2[c * R : (c + 1) * R]) for c in cores]

    # L1: partial sums -> bounds on host
    nc1 = _get("stats1", _build_stats1)
    r1 = _run("stats1", nc1, [{"w_sh": w_shards[c]} for c in cores], cores)
    st1 = np.stack([r1[c]["stats1"] for c in cores]).astype(np.float64)

    s_w = st1[:, :, 0:PO_W].sum()
    s_w2 = st1[:, :, PO_W : 2 * PO_W].sum()
    s_abs = st1[:, :, 2 * PO_W : 3 * PO_W].sum()
    mean = s_w / N_ELEM
    var = (s_w2 - N_ELEM * mean * mean) / (N_ELEM - 1.0)
    std = np.sqrt(var)
    lower = mean - NSIG * std
    upper = mean + NSIG * std

    # L2: outlier masks + masked partials -> scale on host
    nc2 = _get("maskstats", _build_maskstats)
    bnd = np.zeros((128, 2), np.float32)
    bnd[:, 0] = lower
    bnd[:, 1] = upper
    r2 = _run(
        "maskstats", nc2, [{"w_sh": w_shards[c], "bnd": bnd} for c in cores], cores
    )
    st2 = np.stack([r2[c]["stats2"] for c in cores]).astype(np.float64)
    masks = [r2[c]["mask"] for c in cores]
    s_abs_out = st2[:, :, 0:PO_W].sum()
    cnt_out = st2[:, :, PO_W : 2 * PO_W].sum()
    scale = (s_abs - s_abs_out) / (N_ELEM - cnt_out)

    # L3: W_bin shards + x -> bf16
    nc3 = _get("apply", _build_apply)
    sc = np.zeros((128, 2), np.float32)
    sc[:, 0] = scale
    sc[:, 1] = 1.0 - np.float32(scale)
    r3 = _run(
        "apply",
        nc3,
        [
            {"w_sh": w_shards[c], "mask": masks[c], "sc": sc, "x_sh": x_shards[c]}
            for c in cores
        ],
        cores,
    )
    wbin = np.ascontiguousarray(
        np.concatenate([r3[c]["wbin"] for c in cores], axis=0)
    )
    xb_shards = [r3[c]["xb"] for c in cores]

    # L4: data-parallel GEMM
    nc4 = _get("gemm", _build_gemm)
    bias_rep = np.ascontiguousarray(
        np.broadcast_to(bias.astype(np.float32), (128, O))
    )
    r4 = _run(
        "gemm",
        nc4,
        [{"xb": xb_shards[c], "wbin": wbin, "bias_rep": bias_rep} for c in cores],
        cores,
    )
    out = np.concatenate([r4[c]["out"] for c in cores], axis=0)
    return out.reshape(B, S, O)


# revision 13
# speedup vs baseline: 1.1236x; 1.0310x over previous
"""Trainium2 Bass kernel for nn_BinaryXnorExceptOutliersLinear.

Reference computation (fp32):
    mean = mean(W); std = std(W, ddof=1)
    outliers = (W < mean - 1.96 std) | (W > mean + 1.96 std)
    scale = sum(|W| * ~outliers) / sum(~outliers)            # scalar
    W_bin = where(outliers, W, W * scale)
    out   = x @ W_bin.T + bias                               # x: [4, 2048, 4096]

Distribution strategy (8 NeuronCores, full inputs in / full output out):
  * Weight-stat passes are sharded over W rows (512 rows/core). The global
    scalars (mean/std bounds, then scale) are reduced on the host between
    launches -- a few dozen floats each time. Host-mediated beats an
    on-device collective here: PJRT dispatch skews core start times by
    tens of us, and a mid-kernel AllReduce pulls that skew into every
    core's measured span.
  * The GEMM is data-parallel over x rows (1024 rows/core); every core
    reads the full binarized weight (bf16) and computes its slice of the
    output.

Four SPMD launches (identical NEFF on cores 0-7, different per-core data):
  L1 stats1:    per-core partial [sum W, sum W^2, sum |W|] over its W shard.
  L2 maskstats: given (lower, upper): outlier mask shard (bf16 0/1) and
                partial [sum |W*outlier|, count], with the count and masked
                sums fused into the mask-building DVE ops.
  L3 apply:     given scale: W_bin = W * (outlier*(1-scale) + scale) in
                bf16; also casts its x row-shard to bf16.
  L4 gemm:      out_shard = x_shard @ W_bin^T + bias, bf16 matmuls with
                fp32 PSUM accumulation; operand tiles are produced with
                DMA transpose (contraction dim on partitions).

All NEFFs are input-independent (runtime scalars enter as small tensors),
so the neuron compile cache stays warm across runs.
"""

import numpy as np
import ml_dtypes

import concourse.bass as bass
import concourse.mybir as mybir
import concourse.tile as tile
from concourse import bacc
from concourse.bass_utils import run_bass_kernel_spmd
from concourse.kernels.tile_matmul import (
    composable_matmul_tile_kernel,
    dma_from_dram_kxm,
    dma_from_dram_kxn,
    dma_to_dram_mxn,
    lru_cache_producer,
)

F32 = mybir.dt.float32
BF16 = mybir.dt.bfloat16
ALU = mybir.AluOpType

N_CORES = 8
B, S, D = 4, 2048, 4096  # x: [B, S, D]
O = 4096                 # weight: [O, D]
R_TOT = B * S            # 8192 rows total
R = R_TOT // N_CORES     # 1024 x-rows per core
OSH = O // N_CORES       # 512 weight rows per core (stat shards)
PO_W = OSH // 128        # 4 partition-chunks per W shard
PO_X = R // 128          # 8 partition-chunks per x shard
N_ELEM = float(O * D)
NSIG = 1.96

_CACHE = {}
LAST_TIMINGS = []  # (name, exec_time_ns) per launch when BASS_TRACE=1


def _wview(ap):
    return ap.rearrange("(po pi) f -> pi po f", pi=128)


# ---------------------------------------------------------------- L1: stats1
def _build_stats1():
    nc = bacc.Bacc(None, target_bir_lowering=False)
    w_sh = nc.declare_dram_parameter("w_sh", [OSH, D], F32, isOutput=False)
    st_o = nc.declare_dram_parameter("stats1", [128, 3 * PO_W], F32, isOutput=True)

    with tile.TileContext(nc) as tc:
        with (
            tc.tile_pool(name="wp", bufs=1) as wp,
            tc.tile_pool(name="scr", bufs=2) as scr,
        ):
            wv = _wview(w_sh)
            st = wp.tile([128, 3 * PO_W], F32)
            for c in range(PO_W):
                wt = scr.tile([128, D], F32, tag="wt")
                nc.sync.dma_start(wt[:], wv[:, c])
                # sum W (exact fp32 tree on DVE; feeds the outlier threshold)
                nc.vector.tensor_reduce(
                    st[:, c : c + 1], wt[:], axis=mybir.AxisListType.X, op=ALU.add
                )
                # sum W^2: ACT squares, DVE reduces (threshold-sensitive too)
                sq = scr.tile([128, D], F32, tag="sq")
                nc.scalar.activation(sq[:], wt[:], mybir.ActivationFunctionType.Square)
                nc.vector.tensor_reduce(
                    st[:, PO_W + c : PO_W + c + 1],
                    sq[:],
                    axis=mybir.AxisListType.X,
                    op=ALU.add,
                )
                # sum |W| (only feeds `scale`; ACT accumulate is plenty accurate)
                dm = scr.tile([128, D], BF16, tag="dm")
                nc.scalar.activation(
                    dm[:],
                    wt[:],
                    mybir.ActivationFunctionType.Abs,
                    accum_out=st[:, 2 * PO_W + c : 2 * PO_W + c + 1],
                )
            nc.sync.dma_start(st_o[:], st[:])
    nc.compile()
    return nc


# ------------------------------------------------------- L2: mask + stats2
def _build_maskstats():
    nc = bacc.Bacc(None, target_bir_lowering=False)
    w_sh = nc.declare_dram_parameter("w_sh", [OSH, D], F32, isOutput=False)
    bnd = nc.declare_dram_parameter("bnd", [128, 2], F32, isOutput=False)
    st_o = nc.declare_dram_parameter("stats2", [128, 2 * PO_W], F32, isOutput=True)

    with tile.TileContext(nc) as tc:
        with (
            tc.tile_pool(name="cn", bufs=1) as cn,
            tc.tile_pool(name="ch", bufs=2) as ch,
        ):
            bt = cn.tile([128, 2], F32)
            nc.sync.dma_start(bt[:], bnd[:])
            lo_ap = bt[:, 0:1]
            hi_ap = bt[:, 1:2]
            st = cn.tile([128, 2 * PO_W], F32)
            wv = _wview(w_sh)
            for c in range(PO_W):
                wt = ch.tile([128, D], F32, tag="wt")
                nc.sync.dma_start(wt[:], wv[:, c])
                mhi = ch.tile([128, D], F32, tag="mhi")
                nc.vector.tensor_scalar(mhi[:], wt[:], hi_ap, None, ALU.is_gt)
                # outlier = (w < lo) + (w > hi); count fused into accum
                mko = ch.tile([128, D], BF16, tag="mko")
                nc.vector.scalar_tensor_tensor(
                    mko[:],
                    wt[:],
                    lo_ap,
                    mhi[:],
                    ALU.is_lt,
                    ALU.add,
                    accum_out=st[:, PO_W + c : PO_W + c + 1],
                )
                # |w| on ACT, then sum |w|*outlier fused on DVE
                aw = ch.tile([128, D], BF16, tag="aw")
                nc.scalar.activation(aw[:], wt[:], mybir.ActivationFunctionType.Abs)
                dm = ch.tile([128, D], BF16, tag="dm")
                nc.vector.scalar_tensor_tensor(
                    dm[:],
                    aw[:],
                    1.0,
                    mko[:],
                    ALU.mult,
                    ALU.mult,
                    accum_out=st[:, c : c + 1],
                )
            nc.sync.dma_start(st_o[:], st[:])
    nc.compile()
    return nc


# ---------------------------------------------------------------- L3: apply
def _build_apply():
    nc = bacc.Bacc(None, target_bir_lowering=False)
    w_sh = nc.declare_dram_parameter("w_sh", [OSH, D], F32, isOutput=False)
    sc_i = nc.declare_dram_parameter("sc", [128, 4], F32, isOutput=False)
    x_sh = nc.declare_dram_parameter("x_sh", [R, D], F32, isOutput=False)
    wb_o = nc.declare_dram_parameter("wbin", [OSH, D], BF16, isOutput=True)
    xb_o = nc.declare_dram_parameter("xb", [R, D], BF16, isOutput=True)

    with tile.TileContext(nc) as tc:
        with (
            tc.tile_pool(name="cn", bufs=1) as cn,
            tc.tile_pool(name="ch", bufs=2) as ch,
            tc.tile_pool(name="xp", bufs=3) as xp,
        ):
            sct = cn.tile([128, 4], F32)
            nc.sync.dma_start(sct[:], sc_i[:])
            sc0 = sct[:, 0:1]  # scale
            sc1 = sct[:, 1:2]  # 1 - scale
            lo_ap = sct[:, 2:3]
            hi_ap = sct[:, 3:4]
            wv = _wview(w_sh)
            bv = _wview(wb_o)
            for c in range(PO_W):
                wt = ch.tile([128, D], F32, tag="wt")
                nc.sync.dma_start(wt[:], wv[:, c])
                # recompute outlier mask (cheaper than a DRAM round-trip),
                # then f = outlier*(1-scale) + scale in one fused op chain
                mhi = ch.tile([128, D], F32, tag="mhi")
                nc.vector.tensor_scalar(mhi[:], wt[:], hi_ap, None, ALU.is_gt)
                mko = ch.tile([128, D], F32, tag="mko")
                nc.vector.scalar_tensor_tensor(
                    mko[:], wt[:], lo_ap, mhi[:], ALU.is_lt, ALU.add
                )
                f = ch.tile([128, D], F32, tag="f")
                nc.vector.tensor_scalar(f[:], mko[:], sc1, sc0, ALU.mult, ALU.add)
                wb = ch.tile([128, D], BF16, tag="wb")
                nc.vector.tensor_tensor(wb[:], wt[:], f[:], ALU.mult)
                nc.sync.dma_start(bv[:, c], wb[:])

            # x -> bf16 (DVE cast copies; DMA on the scalar HWDGE queue)
            HD = D // 2
            xv = x_sh.rearrange("(po pi) (fo fi) -> pi po fo fi", pi=128, fi=HD)
            xbv = xb_o.rearrange("(po pi) (fo fi) -> pi po fo fi", pi=128, fi=HD)
            for c in range(2 * PO_X):
                xt = xp.tile([128, HD], F32, tag="xt")
                nc.scalar.dma_start(xt[:], xv[:, c // 2, c % 2])
                xbt = xp.tile([128, HD], BF16, tag="xbt")
                nc.vector.tensor_copy(xbt[:], xt[:])
                nc.scalar.dma_start(xbv[:, c // 2, c % 2], xbt[:])
    nc.compile()
    return nc


# ---------------------------------------------------------------- L4: gemm
def _build_gemm():
    nc = bacc.Bacc(None, target_bir_lowering=False)
    xb = nc.declare_dram_parameter("xb", [R, D], BF16, isOutput=False)
    wb = nc.declare_dram_parameter("wbin", [O, D], BF16, isOutput=False)
    br = nc.declare_dram_parameter("bias_cols", [128, O // 128], F32, isOutput=False)
    out = nc.declare_dram_parameter("out", [O, R], F32, isOutput=True)

    M_SUB = 4  # 512 // 128

    with tile.TileContext(nc) as tc:
        import contextlib

        with contextlib.ExitStack() as ctx:
            cn = ctx.enter_context(tc.tile_pool(name="const", bufs=1))
            kxm_pool = ctx.enter_context(tc.tile_pool(name="kxm_pool", bufs=3))
            kxn_pool = ctx.enter_context(tc.tile_pool(name="kxn_pool", bufs=4))

            bias_sb = cn.tile([128, O // 128], F32)
            nc.sync.dma_start(bias_sb[:], br[:])

            # kxm = W_bin^T tiles [k, m] (stationary side; streamed once),
            # kxn = x^T tiles [k, n] (moving side; LRU-cached in SBUF, so
            # the snake re-visits never re-DMA).
            kxm_producer, kxm_shape = dma_from_dram_kxm(
                kxm_pool, wb[:], transpose_ap=True
            )
            kxn_producer, kxn_shape = lru_cache_producer(
                dma_from_dram_kxn(kxn_pool, xb[:], transpose_ap=True),
                buffer_size=4,
            )
            mxn_consumer = dma_to_dram_mxn(out[:])

            def reducer(nc, psum, sbuf, md):
                # bias is per output-feature = per psum partition here
                col = md.m_tile_idx * M_SUB + md.m_subtile_idx
                nss = md.n_slice_size
                nc.vector.tensor_scalar(
                    sbuf[:, 0, :nss],
                    psum[:, :nss],
                    bias_sb[:, col : col + 1],
                    None,
                    ALU.add,
                )

            composable_matmul_tile_kernel(
                tc,
                kxm_shape=kxm_shape,
                kxn_shape=kxn_shape,
                output_type=F32,
                kxm_producer=kxm_producer,
                kxn_producer=kxn_producer,
                mxn_consumer=mxn_consumer,
                mxn_subtile_reducer=reducer,
                psum_n_bufs=2,
                cache_tiles=True,
                MAX_K_TILE_SIZE=2048,
            )
    nc.compile()
    return nc


def _get(name, builder):
    if name not in _CACHE:
        _CACHE[name] = builder()
    return _CACHE[name]


def _run(name, nc, in_maps, cores):
    r = run_bass_kernel_spmd(nc, in_maps, core_ids=cores)
    if r.exec_time_ns is not None:
        LAST_TIMINGS.append((name, r.exec_time_ns))
    return r.results


# ---------------------------------------------------------------- driver
def kernel(x: np.ndarray, weight: np.ndarray, bias: np.ndarray) -> np.ndarray:
    LAST_TIMINGS.clear()
    x2 = np.ascontiguousarray(x.reshape(R_TOT, D))
    w = np.ascontiguousarray(weight)
    cores = list(range(N_CORES))
    w_shards = [np.ascontiguousarray(w[c * OSH : (c + 1) * OSH]) for c in cores]
    x_shards = [np.ascontiguousarray(x2[c * R : (c + 1) * R]) for c in cores]

    # L1: partial sums -> bounds on host
    nc1 = _get("stats1", _build_stats1)
    r1 = _run("stats1", nc1, [{"w_sh": w_shards[c]} for c in cores], cores)
    st1 = np.stack([r1[c]["stats1"] for c in cores]).astype(np.float64)

    s_w = st1[:, :, 0:PO_W].sum()
    s_w2 = st1[:, :, PO_W : 2 * PO_W].sum()
    s_abs = st1[:, :, 2 * PO_W : 3 * PO_W].sum()
    mean = s_w / N_ELEM
    var = (s_w2 - N_ELEM * mean * mean) / (N_ELEM - 1.0)
    std = np.sqrt(var)
    lower = mean - NSIG * std
    upper = mean + NSIG * std

    # L2: outlier masks + masked partials -> scale on host
    nc2 = _get("maskstats", _build_maskstats)
    bnd = np.zeros((128, 2), np.float32)
    bnd[:, 0] = lower
    bnd[:, 1] = upper
    r2 = _run(
        "maskstats", nc2, [{"w_sh": w_shards[c], "bnd": bnd} for c in cores], cores
    )
    st2 = np.stack([r2[c]["stats2"] for c in cores]).astype(np.float64)
    s_abs_out = st2[:, :, 0:PO_W].sum()
    cnt_out = st2[:, :, PO_W : 2 * PO_W].sum()
    scale = (s_abs - s_abs_out) / (N_ELEM - cnt_out)

    # L3: W_bin shards + x -> bf16
    nc3 = _get("apply", _build_apply)
    sc = np.zeros((128, 4), np.float32)
    sc[:, 0] = scale
    sc[:, 1] = 1.0 - np.float32(scale)
    sc[:, 2] = lower
    sc[:, 3] = upper
    r3 = _run(
        "apply",
        nc3,
        [{"w_sh": w_shards[c], "sc": sc, "x_sh": x_shards[c]} for c in cores],
        cores,
    )
    wbin = np.ascontiguousarray(
        np.concatenate([r3[c]["wbin"] for c in cores], axis=0)
    )
    xb_shards = [r3[c]["xb"] for c in cores]

    # L4: data-parallel GEMM
    nc4 = _get("gemm", _build_gemm)
    # bias_cols[p, col] = bias[col*128 + p] (psum partition layout)
    bias_cols = np.ascontiguousarray(
        bias.astype(np.float32).reshape(O // 128, 128).T
    )
    r4 = _run(
        "gemm",
        nc4,
        [{"xb": xb_shards[c], "wbin": wbin, "bias_cols": bias_cols} for c in cores],
        cores,
    )
    # per-core output is [O, R] (features x rows); transpose in the gather
    out = np.empty((R_TOT, O), np.float32)
    for c in cores:
        out[c * R : (c + 1) * R, :] = r4[c]["out"].T
    return out.reshape(B, S, O)


# revision 16
# speedup vs baseline: 1.1283x; 1.0042x over previous
"""Trainium2 Bass kernel for nn_BinaryXnorExceptOutliersLinear.

Reference computation (fp32):
    mean = mean(W); std = std(W, ddof=1)
    outliers = (W < mean - 1.96 std) | (W > mean + 1.96 std)
    scale = sum(|W| * ~outliers) / sum(~outliers)            # scalar
    W_bin = where(outliers, W, W * scale)
    out   = x @ W_bin.T + bias                               # x: [4, 2048, 4096]

Distribution strategy (8 NeuronCores, full inputs in / full output out):
  * Weight-stat passes are sharded over W rows (512 rows/core). The global
    scalars (mean/std bounds, then scale) are reduced on the host between
    launches -- a few dozen floats each time. Host-mediated beats an
    on-device collective here: PJRT dispatch skews core start times by
    tens of us, and a mid-kernel AllReduce pulls that skew into every
    core's measured span.
  * The GEMM is data-parallel over x rows (1024 rows/core); every core
    reads the full binarized weight (bf16) and computes its slice of the
    output.

Four SPMD launches (identical NEFF on cores 0-7, different per-core data):
  L1 stats1:    per-core partial [sum W, sum W^2, sum |W|] over its W shard.
  L2 maskstats: given (lower, upper): outlier mask shard (bf16 0/1) and
                partial [sum |W*outlier|, count], with the count and masked
                sums fused into the mask-building DVE ops.
  L3 apply:     given scale: W_bin = W * (outlier*(1-scale) + scale) in
                bf16; also casts its x row-shard to bf16.
  L4 gemm:      out_shard = x_shard @ W_bin^T + bias, bf16 matmuls with
                fp32 PSUM accumulation; operand tiles are produced with
                DMA transpose (contraction dim on partitions).

All NEFFs are input-independent (runtime scalars enter as small tensors),
so the neuron compile cache stays warm across runs.
"""

import numpy as np
import ml_dtypes

import concourse.bass as bass
import concourse.mybir as mybir
import concourse.tile as tile
from concourse import bacc
from concourse.bass_utils import run_bass_kernel_spmd
from concourse.kernels.tile_matmul import (
    composable_matmul_tile_kernel,
    dma_from_dram_kxm,
    dma_from_dram_kxn,
    dma_to_dram_mxn,
    lru_cache_producer,
)

F32 = mybir.dt.float32
BF16 = mybir.dt.bfloat16
ALU = mybir.AluOpType

N_CORES = 8
B, S, D = 4, 2048, 4096  # x: [B, S, D]
O = 4096                 # weight: [O, D]
R_TOT = B * S            # 8192 rows total
R = R_TOT // N_CORES     # 1024 x-rows per core
OSH = O // N_CORES       # 512 weight rows per core (stat shards)
PO_W = OSH // 128        # 4 partition-chunks per W shard
PO_X = R // 128          # 8 partition-chunks per x shard
N_ELEM = float(O * D)
NSIG = 1.96

_CACHE = {}
LAST_TIMINGS = []  # (name, exec_time_ns) per launch when BASS_TRACE=1


def _wview(ap):
    return ap.rearrange("(po pi) f -> pi po f", pi=128)


# ---------------------------------------------------------------- L1: stats1
def _build_stats1():
    nc = bacc.Bacc(None, target_bir_lowering=False)
    w_sh = nc.declare_dram_parameter("w_sh", [OSH, D], F32, isOutput=False)
    st_o = nc.declare_dram_parameter("stats1", [128, 3 * PO_W], F32, isOutput=True)

    with tile.TileContext(nc) as tc:
        with (
            tc.tile_pool(name="wp", bufs=1) as wp,
            tc.tile_pool(name="scr", bufs=2) as scr,
        ):
            wv = _wview(w_sh)
            st = wp.tile([128, 3 * PO_W], F32)
            for c in range(PO_W):
                wt = scr.tile([128, D], F32, tag="wt")
                nc.sync.dma_start(wt[:], wv[:, c])
                # sum W (exact fp32 tree on DVE; feeds the outlier threshold)
                nc.vector.tensor_reduce(
                    st[:, c : c + 1], wt[:], axis=mybir.AxisListType.X, op=ALU.add
                )
                # sum W^2: ACT squares, DVE reduces (threshold-sensitive too)
                sq = scr.tile([128, D], F32, tag="sq")
                nc.scalar.activation(sq[:], wt[:], mybir.ActivationFunctionType.Square)
                nc.vector.tensor_reduce(
                    st[:, PO_W + c : PO_W + c + 1],
                    sq[:],
                    axis=mybir.AxisListType.X,
                    op=ALU.add,
                )
                # sum |W| (only feeds `scale`; ACT accumulate is plenty accurate)
                dm = scr.tile([128, D], BF16, tag="dm")
                nc.scalar.activation(
                    dm[:],
                    wt[:],
                    mybir.ActivationFunctionType.Abs,
                    accum_out=st[:, 2 * PO_W + c : 2 * PO_W + c + 1],
                )
            nc.sync.dma_start(st_o[:], st[:])
    nc.compile()
    return nc


# ------------------------------------------------------- L2: mask + stats2
def _build_maskstats():
    nc = bacc.Bacc(None, target_bir_lowering=False)
    w_sh = nc.declare_dram_parameter("w_sh", [OSH, D], F32, isOutput=False)
    bnd = nc.declare_dram_parameter("bnd", [128, 2], F32, isOutput=False)
    x_sh = nc.declare_dram_parameter("x_sh", [R, D], F32, isOutput=False)
    st_o = nc.declare_dram_parameter("stats2", [128, 2 * PO_W], F32, isOutput=True)
    xb_o = nc.declare_dram_parameter("xb", [R, D], BF16, isOutput=True)

    with tile.TileContext(nc) as tc:
        with (
            tc.tile_pool(name="cn", bufs=1) as cn,
            tc.tile_pool(name="ch", bufs=2) as ch,
            tc.tile_pool(name="xp", bufs=3) as xp,
        ):
            bt = cn.tile([128, 2], F32)
            nc.sync.dma_start(bt[:], bnd[:])
            lo_ap = bt[:, 0:1]
            hi_ap = bt[:, 1:2]
            st = cn.tile([128, 2 * PO_W], F32)
            wv = _wview(w_sh)
            for c in range(PO_W):
                wt = ch.tile([128, D], F32, tag="wt")
                nc.sync.dma_start(wt[:], wv[:, c])
                mhi = ch.tile([128, D], F32, tag="mhi")
                nc.vector.tensor_scalar(mhi[:], wt[:], hi_ap, None, ALU.is_gt)
                # outlier = (w < lo) + (w > hi); count fused into accum
                mko = ch.tile([128, D], BF16, tag="mko")
                nc.vector.scalar_tensor_tensor(
                    mko[:],
                    wt[:],
                    lo_ap,
                    mhi[:],
                    ALU.is_lt,
                    ALU.add,
                    accum_out=st[:, PO_W + c : PO_W + c + 1],
                )
                # |w| on ACT, then sum |w|*outlier fused on DVE
                aw = ch.tile([128, D], BF16, tag="aw")
                nc.scalar.activation(aw[:], wt[:], mybir.ActivationFunctionType.Abs)
                dm = ch.tile([128, D], BF16, tag="dm")
                nc.vector.scalar_tensor_tensor(
                    dm[:],
                    aw[:],
                    1.0,
                    mko[:],
                    ALU.mult,
                    ALU.mult,
                    accum_out=st[:, c : c + 1],
                )
            nc.sync.dma_start(st_o[:], st[:])

            # x -> bf16 (DVE cast copies; DMA on the scalar HWDGE queue)
            HD = D // 2
            xv = x_sh.rearrange("(po pi) (fo fi) -> pi po fo fi", pi=128, fi=HD)
            xbv = xb_o.rearrange("(po pi) (fo fi) -> pi po fo fi", pi=128, fi=HD)
            for c in range(2 * PO_X):
                xt = xp.tile([128, HD], F32, tag="xt")
                nc.scalar.dma_start(xt[:], xv[:, c // 2, c % 2])
                xbt = xp.tile([128, HD], BF16, tag="xbt")
                nc.vector.tensor_copy(xbt[:], xt[:])
                nc.scalar.dma_start(xbv[:, c // 2, c % 2], xbt[:])
    nc.compile()
    return nc


# ------------------------------------------------- L3: apply + gemm (fused)
def _build_applygemm():
    """Column-parallel: this core binarizes its own W shard (512 out rows)
    on-chip, bounces it through internal DRAM to get the contraction dim
    onto partitions via DMA transpose, then computes out^T[o_shard, :] =
    W_bin_shard @ x^T for ALL 8192 x rows (streamed once, bf16)."""
    nc = bacc.Bacc(None, target_bir_lowering=False)
    w_sh = nc.declare_dram_parameter("w_sh", [OSH, D], F32, isOutput=False)
    sc_i = nc.declare_dram_parameter("sc", [128, 4], F32, isOutput=False)
    xb = nc.declare_dram_parameter("xb", [R_TOT, D], BF16, isOutput=False)
    br = nc.declare_dram_parameter("bias_cols", [128, PO_W], F32, isOutput=False)
    out = nc.declare_dram_parameter("out", [OSH, R_TOT], F32, isOutput=True)

    with tile.TileContext(nc) as tc:
        import contextlib

        with contextlib.ExitStack() as ctx:
            cn = ctx.enter_context(tc.tile_pool(name="cn", bufs=1))
            ch = ctx.enter_context(tc.tile_pool(name="ch", bufs=2))
            dramp = ctx.enter_context(tc.tile_pool(name="dram", bufs=1, space="DRAM"))
            kxm_pool = ctx.enter_context(tc.tile_pool(name="kxm_pool", bufs=5))
            kxn_pool = ctx.enter_context(tc.tile_pool(name="kxn_pool", bufs=4))

            sct = cn.tile([128, 4], F32)
            nc.sync.dma_start(sct[:], sc_i[:])
            sc0 = sct[:, 0:1]  # scale
            sc1 = sct[:, 1:2]  # 1 - scale
            lo_ap = sct[:, 2:3]
            hi_ap = sct[:, 3:4]
            bias_sb = cn.tile([128, PO_W], F32)
            nc.sync.dma_start(bias_sb[:], br[:])

            wbin = dramp.tile([OSH, D], BF16)
            wv = _wview(w_sh)
            bv = _wview(wbin[:])
            for c in range(PO_W):
                wt = ch.tile([128, D], F32, tag="wt")
                nc.sync.dma_start(wt[:], wv[:, c])
                mhi = ch.tile([128, D], F32, tag="mhi")
                nc.vector.tensor_scalar(mhi[:], wt[:], hi_ap, None, ALU.is_gt)
                # mask, then f = mask*(1-scale)+scale, in place on mhi
                nc.vector.scalar_tensor_tensor(
                    mhi[:], wt[:], lo_ap, mhi[:], ALU.is_lt, ALU.add
                )
                nc.vector.tensor_scalar(mhi[:], mhi[:], sc1, sc0, ALU.mult, ALU.add)
                wb = ch.tile([128, D], BF16, tag="wb")
                nc.vector.tensor_tensor(wb[:], wt[:], mhi[:], ALU.mult)
                nc.sync.dma_start(bv[:, c], wb[:])

            # kxm = W_bin^T tiles (this core's 512 features; stays cached),
            # kxn = x^T tiles, full 8192 rows streamed exactly once.
            kxm_producer, kxm_shape = dma_from_dram_kxm(
                kxm_pool, wbin[:], transpose_ap=True
            )
            kxn_producer, kxn_shape = dma_from_dram_kxn(
                kxn_pool, xb[:], transpose_ap=True
            )
            mxn_consumer = dma_to_dram_mxn(out[:])

            def reducer(nc, psum, sbuf, md):
                # bias per output-feature = per psum partition
                col = md.m_tile_idx * PO_W + md.m_subtile_idx
                nss = md.n_slice_size
                nc.vector.tensor_scalar(
                    sbuf[:, 0, :nss],
                    psum[:, :nss],
                    bias_sb[:, col : col + 1],
                    None,
                    ALU.add,
                )

            composable_matmul_tile_kernel(
                tc,
                kxm_shape=kxm_shape,
                kxn_shape=kxn_shape,
                output_type=F32,
                kxm_producer=kxm_producer,
                kxn_producer=kxn_producer,
                mxn_consumer=mxn_consumer,
                mxn_subtile_reducer=reducer,
                psum_n_bufs=2,
                cache_tiles=True,
                MAX_K_TILE_SIZE=1024,
            )
    nc.compile()
    return nc


def _get(name, builder):
    if name not in _CACHE:
        _CACHE[name] = builder()
    return _CACHE[name]


def _run(name, nc, in_maps, cores):
    r = run_bass_kernel_spmd(nc, in_maps, core_ids=cores)
    if r.exec_time_ns is not None:
        LAST_TIMINGS.append((name, r.exec_time_ns))
    return r.results


# ---------------------------------------------------------------- driver
def kernel(x: np.ndarray, weight: np.ndarray, bias: np.ndarray) -> np.ndarray:
    LAST_TIMINGS.clear()
    x2 = np.ascontiguousarray(x.reshape(R_TOT, D))
    w = np.ascontiguousarray(weight)
    cores = list(range(N_CORES))
    w_shards = [np.ascontiguousarray(w[c * OSH : (c + 1) * OSH]) for c in cores]
    x_shards = [np.ascontiguousarray(x2[c * R : (c + 1) * R]) for c in cores]

    # L1: partial sums -> bounds on host
    nc1 = _get("stats1", _build_stats1)
    r1 = _run("stats1", nc1, [{"w_sh": w_shards[c]} for c in cores], cores)
    st1 = np.stack([r1[c]["stats1"] for c in cores]).astype(np.float64)

    s_w = st1[:, :, 0:PO_W].sum()
    s_w2 = st1[:, :, PO_W : 2 * PO_W].sum()
    s_abs = st1[:, :, 2 * PO_W : 3 * PO_W].sum()
    mean = s_w / N_ELEM
    var = (s_w2 - N_ELEM * mean * mean) / (N_ELEM - 1.0)
    std = np.sqrt(var)
    lower = mean - NSIG * std
    upper = mean + NSIG * std

    # L2: masked partials (-> scale on host) + x cast to bf16
    nc2 = _get("maskstats", _build_maskstats)
    bnd = np.zeros((128, 2), np.float32)
    bnd[:, 0] = lower
    bnd[:, 1] = upper
    r2 = _run(
        "maskstats",
        nc2,
        [{"w_sh": w_shards[c], "bnd": bnd, "x_sh": x_shards[c]} for c in cores],
        cores,
    )
    st2 = np.stack([r2[c]["stats2"] for c in cores]).astype(np.float64)
    s_abs_out = st2[:, :, 0:PO_W].sum()
    cnt_out = st2[:, :, PO_W : 2 * PO_W].sum()
    scale = (s_abs - s_abs_out) / (N_ELEM - cnt_out)
    xb_full = np.ascontiguousarray(
        np.concatenate([r2[c]["xb"] for c in cores], axis=0)
    )

    # L3: fused binarize + column-parallel GEMM
    nc3 = _get("applygemm", _build_applygemm)
    sc = np.zeros((128, 4), np.float32)
    sc[:, 0] = scale
    sc[:, 1] = 1.0 - np.float32(scale)
    sc[:, 2] = lower
    sc[:, 3] = upper
    bias32 = bias.astype(np.float32)
    r3 = _run(
        "applygemm",
        nc3,
        [
            {
                "w_sh": w_shards[c],
                "sc": sc,
                "xb": xb_full,
                "bias_cols": np.ascontiguousarray(
                    bias32[c * OSH : (c + 1) * OSH].reshape(PO_W, 128).T
                ),
            }
            for c in cores
        ],
        cores,
    )
    # per-core output is [OSH, R_TOT] = (features x rows); gather + transpose
    out = np.empty((R_TOT, O), np.float32)
    for c in cores:
        out[:, c * OSH : (c + 1) * OSH] = r3[c]["out"].T
    return out.reshape(B, S, O)
